# revision 1
# baseline (speedup 1.0000x reference)
"""DAHH hypergraph conv (gnn_message_passing) on 8 Trainium2 NeuronCores.

reference:
    xp      = x @ theta                      [N, 64]
    de      = colsum(H)                      [E]
    edge_ft = (H.T @ xp) / de[:, None]       [E, 64]
    dn      = rowsum(H)                      [N]
    node_ft = (H @ edge_ft) / dn[:, None]    [N, 64]

Sharding: H and x row-sharded (node dim) across 8 cores; theta replicated.
Per core:
  phase 0: xp1[k] = [x_shard @ theta | 1]  (PE transpose of x tiles + matmul)
  phase 1: partial edge sums  P = H_shard.T @ xp1  as [65, e] psum chunks
           (lhsT=xp1 bf16, rhs=H cast to bf16 on DVE/ACT, full PE rate),
           transposed on PE into [e, 65] tiles, staged to DRAM bounce.
  phase 2: AllReduce (sum partial edge sums over the 8 node shards).
  phase 3: edge_ft1[e] = [P[e,0:64]/max(P[e,64],eps) | 1]  (per-partition
           scalar ops; edges on partitions).
  phase 4: node out: po = sum_e H.T_tile[e,n].T @ eft1[e]  -> [n, 65];
           H tiles PE-transposed on the fly (f32), cast to bf16 for the
           matmul (FWL weight loads); out = po[:,0:64]/max(po[:,64],eps).

All H traffic is HWDGE f32 (measured ~3x faster than SWDGE cast DMA).
N padded 20000->20480 (2560/core), E padded 10000->10240 with zeros; padding
is numerically inert (zero rows/cols, degree clamps via max(.,1e-20)).
"""
import sys
sys.path.insert(0, "/opt/trn_rl_repo")
import numpy as np

import concourse.bass as bass
import concourse.bacc as bacc
import concourse.tile as tile
import concourse.mybir as mybir
from concourse.bass_utils import run_bass_kernel_spmd

N, E, IN_CH, OUT_CH = 20000, 10000, 128, 64
N_CORES = 8
NL = 2560            # padded nodes per core
EP = 10240           # padded edges
NT = NL // 128       # 20 node tiles per core
ET = EP // 128       # 80 edge tiles
CC = 2048            # H DMA chunk width (e cols)
NCC = EP // CC       # 5
W = 65               # 64 features + degree/ones column

f32 = mybir.dt.float32
f32r = mybir.dt.float32r
bf16 = mybir.dt.bfloat16

SKIP_COLLECTIVE = False   # dev-only: replace AllReduce with local copy


def build_body(nc, tc, x_ext, H_ext, th_ext, id_ext, out_ext, sfx="",
               phases=(0, 1, 2, 3, 4), dma_only=False):
    with (
        tc.tile_pool(name=f"const{sfx}", bufs=1) as constp,
        tc.tile_pool(name=f"persist{sfx}", bufs=1) as persist,
        tc.tile_pool(name=f"dram{sfx}", bufs=1, space="DRAM") as dram,
    ):
        ident = constp.tile([128, 128], f32)
        nc.sync.dma_start(ident[:], id_ext[:])
        th_f = constp.tile([128, OUT_CH], f32)
        nc.sync.dma_start(th_f[:], th_ext[:])
        th_b = constp.tile([128, OUT_CH], bf16)
        nc.vector.tensor_copy(th_b[:], th_f[:])
        acc = constp.tile([128, 1], f32)   # dma_only sink
        nc.vector.memset(acc[:], 0.0)

        xp1 = persist.tile([128, NT * W], bf16)
        eft1 = persist.tile([128, ET * W], bf16)
        bounce_in = dram.tile([128, ET * W], f32)
        bounce_out = dram.tile([128, ET * W], f32, addr_space="Shared")

        # ---- phase 0: xp1 = [x @ theta | 1] per node tile ----
        if 0 in phases:
            with (
                tc.tile_pool(name=f"p0{sfx}", bufs=3) as p0,
                tc.tile_pool(name=f"p0ps{sfx}", bufs=2, space="PSUM") as p0ps,
            ):
                for k in range(NT):
                    xt = p0.tile([128, 128], f32, tag="x")
                    nc.sync.dma_start(xt[:], x_ext[k * 128:(k + 1) * 128, :])
                    pt = p0ps.tile([128, 128], f32, tag="pt")
                    nc.tensor.transpose(pt[:], xt[:], ident[:])
                    xT = p0.tile([128, 128], bf16, tag="xT")
                    nc.vector.tensor_copy(xT[:], pt[:])
                    pxp = p0ps.tile([128, OUT_CH], f32, tag="pxp")
                    nc.tensor.matmul(pxp[:], xT[:], th_b[:], start=True, stop=True)
                    nc.vector.tensor_copy(xp1[:, k * W:k * W + OUT_CH], pxp[:])
                    nc.vector.memset(xp1[:, k * W + OUT_CH:(k + 1) * W], 1.0)

        # ---- phase 1: partial edge sums, transposed to [e, 65] tiles ----
        if 1 in phases:
            with (
                tc.tile_pool(name=f"p1stage{sfx}", bufs=1) as p1stage,
                tc.tile_pool(name=f"p1h{sfx}", bufs=5) as p1h,
                tc.tile_pool(name=f"p1ps{sfx}", bufs=1, space="PSUM") as p1ps,
                tc.tile_pool(name=f"p1e{sfx}", bufs=3) as p1e,
                tc.tile_pool(name=f"p1ps2{sfx}", bufs=2, space="PSUM") as p1ps2,
            ):
                ar_in = p1stage.tile([128, ET * W], f32)
                for cc in range(NCC):
                    psums = [p1ps.tile([W, 512], f32, tag=f"pch{j}",
                                       name=f"pch{j}_{cc}{sfx}")
                             for j in range(4)]
                    for k in range(NT):
                        h = p1h.tile([128, CC], f32, tag="h")
                        nc.sync.dma_start(
                            h[:], H_ext[k * 128:(k + 1) * 128, cc * CC:(cc + 1) * CC])
                        if dma_only:
                            nc.vector.tensor_tensor(
                                acc[:], acc[:], h[:, 0:1], mybir.AluOpType.add)
                            continue
                        hb = p1h.tile([128, CC], bf16, tag="hb")
                        if k % 2 == 0:
                            nc.vector.tensor_copy(hb[:], h[:])
                        else:
                            nc.scalar.activation(
                                hb[:], h[:], mybir.ActivationFunctionType.Copy)
                        for j in range(4):
                            nc.tensor.matmul(
                                psums[j][:],
                                xp1[:, k * W:(k + 1) * W],
                                hb[:, j * 512:(j + 1) * 512],
                                start=(k == 0), stop=(k == NT - 1))
                    if dma_only:
                        continue
                    for j in range(4):
                        et = p1e.tile([W, 512], f32, tag="et")
                        nc.vector.tensor_copy(et[:], psums[j][:])
                        for q in range(4):
                            t_idx = cc * 16 + j * 4 + q
                            ptr = p1ps2.tile([128, W], f32, tag="ptr")
                            nc.tensor.transpose(
                                ptr[:], et[:, q * 128:(q + 1) * 128], ident[0:W, 0:W])
                            nc.vector.tensor_copy(
                                ar_in[:, t_idx * W:(t_idx + 1) * W], ptr[:])
                if not dma_only:
                    nc.sync.dma_start(bounce_in[:], ar_in[:])

        # ---- phase 2: AllReduce over the 8 node shards ----
        if 2 in phases:
            if SKIP_COLLECTIVE:
                nc.sync.dma_start(bounce_out[:], bounce_in[:])
            else:
                nc.gpsimd.collective_compute(
                    "AllReduce", mybir.AluOpType.add,
                    replica_groups=[list(range(N_CORES))],
                    ins=[bounce_in.opt()], outs=[bounce_out.opt()])

        # ---- phase 3: normalize edge features; append ones column ----
        if 3 in phases:
            with (
                tc.tile_pool(name=f"p3stage{sfx}", bufs=1) as p3stage,
                tc.tile_pool(name=f"p3{sfx}", bufs=2) as p3,
            ):
                eftf = p3stage.tile([128, ET * W], f32)
                if 2 in phases:
                    nc.sync.dma_start(eftf[:], bounce_out[:])
                else:
                    nc.vector.memset(eftf[:], 1.0)
                for t in range(ET):
                    b = t * W
                    de = p3.tile([128, 1], f32, tag="de")
                    nc.vector.tensor_scalar_max(de[:], eftf[:, b + OUT_CH:b + W], 1e-20)
                    rec = p3.tile([128, 1], f32, tag="rec")
                    nc.vector.reciprocal(rec[:], de[:])
                    nc.vector.tensor_scalar_mul(
                        eft1[:, b:b + OUT_CH], eftf[:, b:b + OUT_CH], rec[:])
                    nc.vector.memset(eft1[:, b + OUT_CH:b + W], 1.0)

        # ---- phase 4: node aggregation with on-the-fly H transpose ----
        if 4 in phases:
            with (
                tc.tile_pool(name=f"p4h{sfx}", bufs=4) as p4h,
                tc.tile_pool(name=f"p4t{sfx}", bufs=32) as p4t,
                tc.tile_pool(name=f"p4ps{sfx}", bufs=3, space="PSUM") as p4ps,
                tc.tile_pool(name=f"p4po{sfx}", bufs=2, space="PSUM") as p4po,
                tc.tile_pool(name=f"p4o{sfx}", bufs=3) as p4o,
            ):
                for nt in range(NT):
                    po = p4po.tile([128, W], f32, tag="po")
                    for cc in range(NCC):
                        h = p4h.tile([128, CC], f32, tag="h")
                        nc.sync.dma_start(
                            h[:], H_ext[nt * 128:(nt + 1) * 128, cc * CC:(cc + 1) * CC])
                        if dma_only:
                            nc.vector.tensor_tensor(
                                acc[:], acc[:], h[:, 0:1], mybir.AluOpType.add)
                            continue
                        for g in range(4):
                            ptr = p4ps.tile([128, 512], f32, tag="ptr")
                            for q in range(4):
                                nc.tensor.transpose(
                                    ptr[:, q * 128:(q + 1) * 128],
                                    h[:, (g * 4 + q) * 128:(g * 4 + q + 1) * 128],
                                    ident[:])
                            hT = p4t.tile([128, 512], bf16, tag="hT")
                            if g % 2 == 0:
                                nc.vector.tensor_copy(hT[:], ptr[:])
                            else:
                                nc.scalar.activation(
                                    hT[:], ptr[:], mybir.ActivationFunctionType.Copy)
                            for q in range(4):
                                t_idx = cc * 16 + g * 4 + q
                                first = (cc == 0 and g == 0 and q == 0)
                                last = (cc == NCC - 1 and g == 3 and q == 3)
                                nc.tensor.matmul(
                                    po[:],
                                    hT[:, q * 128:(q + 1) * 128],
                                    eft1[:, t_idx * W:(t_idx + 1) * W],
                                    start=first, stop=last, skip_group_check=True)
                    if dma_only:
                        continue
                    dn = p4o.tile([128, 1], f32, tag="dn")
                    nc.vector.tensor_scalar_max(dn[:], po[:, OUT_CH:W], 1e-20)
                    rec = p4o.tile([128, 1], f32, tag="rec")
                    nc.vector.reciprocal(rec[:], dn[:])
                    ot = p4o.tile([128, OUT_CH], f32, tag="ot")
                    nc.vector.tensor_scalar_mul(ot[:], po[:, 0:OUT_CH], rec[:])
                    nc.sync.dma_start(out_ext[nt * 128:(nt + 1) * 128, :], ot[:])

        if dma_only or 4 not in phases:
            # make sure something reaches the output so nothing is DCE'd
            ot = constp.tile([128, OUT_CH], f32)
            nc.vector.memset(ot[:], 0.0)
            nc.vector.tensor_tensor(
                ot[:, 0:1], ot[:, 0:1], acc[:], mybir.AluOpType.add)
            if 3 in phases and not dma_only:
                nc.vector.tensor_tensor(
                    ot[:, 0:1], ot[:, 0:1], eft1[:, 0:1], mybir.AluOpType.add)
            if 1 in phases and not dma_only and 3 not in phases:
                nc.vector.tensor_tensor(
                    ot[:, 0:1], ot[:, 0:1], xp1[:, 0:1], mybir.AluOpType.add)
            nc.sync.dma_start(out_ext[0:128, :], ot[:])


def build_graph(reps=1, phases=(0, 1, 2, 3, 4), dma_only=False):
    nc = bacc.Bacc("TRN2", target_bir_lowering=False, debug=False,
                   num_devices=N_CORES)
    x_ext = nc.dram_tensor("x", [NL, IN_CH], f32, kind="ExternalInput")
    H_ext = nc.dram_tensor("H", [NL, EP], f32, kind="ExternalInput")
    th_ext = nc.dram_tensor("theta", [IN_CH, OUT_CH], f32, kind="ExternalInput")
    id_ext = nc.dram_tensor("ident", [128, 128], f32, kind="ExternalInput")
    out_ext = nc.dram_tensor("out", [NL, OUT_CH], f32, kind="ExternalOutput")
    with tile.TileContext(nc) as tc:
        for r in range(reps):
            build_body(nc, tc, x_ext, H_ext, th_ext, id_ext, out_ext,
                       sfx=str(r), phases=phases, dma_only=dma_only)
    nc.compile()
    return nc


def make_in_maps(x, H, theta):
    x_pad = np.zeros((NL * N_CORES, IN_CH), np.float32)
    x_pad[:N] = x
    H_pad = np.zeros((NL * N_CORES, EP), np.float32)
    H_pad[:N, :E] = H
    ident = np.eye(128, dtype=np.float32)
    theta = np.asarray(theta, np.float32)
    in_maps = []
    for c in range(N_CORES):
        in_maps.append({
            "x": x_pad[c * NL:(c + 1) * NL],
            "H": H_pad[c * NL:(c + 1) * NL],
            "theta": theta,
            "ident": ident,
        })
    return in_maps


def kernel(x, H, theta):
    x = np.asarray(x, np.float32)
    H = np.asarray(H, np.float32)
    nc = build_graph(reps=1)
    in_maps = make_in_maps(x, H, theta)
    res = run_bass_kernel_spmd(nc, in_maps, core_ids=list(range(N_CORES)))
    out = np.concatenate(
        [res.results[c]["out"] for c in range(N_CORES)], axis=0)
    return np.ascontiguousarray(out[:N])


if __name__ == "__main__":
    rng = np.random.default_rng(0)
    x = rng.standard_normal((N, IN_CH), dtype=np.float32)
    H = rng.random((N, E), dtype=np.float32)
    theta = (rng.standard_normal((IN_CH, OUT_CH), dtype=np.float32)
             / np.sqrt(IN_CH))
    out = kernel(x, H, theta)
    xp = x @ theta
    de = H.sum(0)
    eft = (H.T @ xp) / de[:, None]
    dn = H.sum(1)
    ref = (H @ eft) / dn[:, None]
    err = np.abs(out - ref).max() / np.abs(ref).max()
    print("rel err:", err)



# revision 3
# speedup vs baseline: 2.6948x; 2.6948x over previous
"""DAHH hypergraph conv v2: single H read, fused transpose production,
chunked pipelined AllReduce.

reference:
    xp      = x @ theta                      [N, 64]
    de      = colsum(H)                      [E]
    edge_ft = (H.T @ xp) / de[:, None]       [E, 64]
    dn      = rowsum(H)                      [N]
    node_ft = (H @ edge_ft) / dn[:, None]    [N, 64]

Sharding: H and x row-sharded (node dim) across 8 cores; theta replicated.

v2 structure (vs baseline's 5 serial phases): the edge dim is split into
NCC=10 chunks of CC=1024. Per chunk cc, stage A loads H[:, cc] once
(f32 HWDGE), casts to bf16, and uses the SBUF-resident tiles TWICE:
(1) edge partial sums  pch = xp1.T @ hb  (psum [65, 512] x2), and
(2) PE transposes of every [128,128] subtile -> hT chunk buffer (bf16),
which stage B consumes as matmul weights — so H is read from HBM ONCE
(~105 MB/core instead of 210 MB). Each chunk's partial edge sums are
AllReduced independently (10 x 266KB collectives); stage B(cc-1) is
issued after stage A(cc) so each collective hides behind the next
chunk's compute. Node aggregation accumulates po (psum per chunk) into
an SBUF accumulator, normalized once at the end.

N padded 20000->20480 (2560/core), E padded 10000->10240 with zeros;
padding is numerically inert (zero rows/cols, degree clamps).
"""
import sys
sys.path.insert(0, "/opt/trn_rl_repo")
import numpy as np

import concourse.bass as bass
import concourse.bacc as bacc
import concourse.tile as tile
import concourse.mybir as mybir
from concourse.bass_utils import run_bass_kernel_spmd

N, E, IN_CH, OUT_CH = 20000, 10000, 128, 64
N_CORES = 8
NL = 2560            # padded nodes per core
EP = 10240           # padded edges
NT = NL // 128       # 20 node tiles per core
CC = 1024            # e-chunk width
NCC = EP // CC       # 10 chunks
ETC = CC // 128      # 8 e-tiles per chunk
W = 65               # 64 features + degree/ones column

f32 = mybir.dt.float32
bf16 = mybir.dt.bfloat16

SKIP_COLLECTIVE = False


def build_body(nc, tc, x_ext, H_ext, th_ext, id_ext, out_ext, sfx="",
               dma_only=False):
    with (
        tc.tile_pool(name=f"const{sfx}", bufs=1) as constp,
        tc.tile_pool(name=f"persist{sfx}", bufs=1) as persist,
        tc.tile_pool(name=f"dram{sfx}", bufs=1, space="DRAM") as dram,
        tc.tile_pool(name=f"hpool{sfx}", bufs=4) as hpool,
        tc.tile_pool(name=f"hbpool{sfx}", bufs=3) as hbpool,
        tc.tile_pool(name=f"evac{sfx}", bufs=2) as evacp,
        tc.tile_pool(name=f"eft{sfx}", bufs=2) as eftp,
        tc.tile_pool(name=f"arin{sfx}", bufs=2) as arinp,
    ):
        ident = constp.tile([128, 128], f32)
        nc.scalar.dma_start(ident[:], id_ext[:])
        ident_b = constp.tile([128, 128], bf16)
        nc.vector.tensor_copy(ident_b[:], ident[:])
        th_f = constp.tile([128, OUT_CH], f32)
        nc.scalar.dma_start(th_f[:], th_ext[:])
        th_b = constp.tile([128, OUT_CH], bf16)
        nc.vector.tensor_copy(th_b[:], th_f[:])
        acc = constp.tile([128, 1], f32)   # dma_only sink
        nc.vector.memset(acc[:], 0.0)

        xp1 = persist.tile([128, NT * W], bf16)
        hT = [persist.tile([128, ETC * NT * 128], bf16, name=f"hT{i}{sfx}")
              for i in range(2)]
        po_acc = persist.tile([128, NT * W], f32)
        bounce_in = [dram.tile([128, ETC * W], f32, name=f"bin{i}{sfx}")
                     for i in range(NCC)]
        bounce_out = [dram.tile([128, ETC * W], f32, addr_space="Shared",
                                name=f"bout{i}{sfx}")
                      for i in range(NCC)]

        # ---- phase 0: xp1 = [x @ theta | 1] per node tile ----
        with (
            tc.tile_pool(name=f"p0{sfx}", bufs=3) as p0,
            tc.tile_pool(name=f"p0ps{sfx}", bufs=2, space="PSUM") as p0ps,
        ):
            for k in range(NT):
                xt = p0.tile([128, 128], f32, tag="x")
                nc.scalar.dma_start(xt[:], x_ext[k * 128:(k + 1) * 128, :])
                pt = p0ps.tile([128, 128], f32, tag="pt")
                nc.tensor.transpose(pt[:], xt[:], ident[:])
                xT = p0.tile([128, 128], bf16, tag="xT")
                nc.vector.tensor_copy(xT[:], pt[:])
                pxp = p0ps.tile([128, OUT_CH], f32, tag="pxp")
                nc.tensor.matmul(pxp[:], xT[:], th_b[:], start=True, stop=True)
                nc.vector.tensor_copy(xp1[:, k * W:k * W + OUT_CH], pxp[:])
                nc.vector.memset(xp1[:, k * W + OUT_CH:(k + 1) * W], 1.0)

        def stage_a(cc, pools):
            pchp, ptrp, ptr2p = pools['pchp'], pools['ptrp'], pools['ptr2p']
            buf = cc % 2
            pchs = [pchp.tile([W, 512], f32, tag=f"pch{j}",
                              name=f"pch{j}_{cc}{sfx}")
                    for j in range(CC // 512)]
            for k in range(NT):
                h = hpool.tile([128, CC], f32, tag="h")
                nc.sync.dma_start(
                    h[:], H_ext[k * 128:(k + 1) * 128, cc * CC:(cc + 1) * CC])
                if dma_only:
                    nc.vector.tensor_tensor(
                        acc[:], acc[:], h[:, 0:1], mybir.AluOpType.add)
                    continue
                hb = hbpool.tile([128, CC], bf16, tag="hb")
                if k % 2 == 0:
                    nc.vector.tensor_copy(hb[:], h[:])
                else:
                    nc.scalar.activation(
                        hb[:], h[:], mybir.ActivationFunctionType.Copy)
                for j in range(CC // 512):
                    nc.tensor.matmul(
                        pchs[j][:],
                        xp1[:, k * W:(k + 1) * W],
                        hb[:, j * 512:(j + 1) * 512],
                        start=(k == 0), stop=(k == NT - 1),
                        skip_group_check=True)
                # transposes for stage B: hT[buf] tile (k, t) at col
                # (k*ETC+t)*128, grouped 4 per [128, 512] bf16 psum
                for g in range(ETC // 4):
                    ptr = ptrp.tile([128, 512], bf16, tag="ptr")
                    for q in range(4):
                        t = g * 4 + q
                        nc.tensor.transpose(
                            ptr[:, q * 128:(q + 1) * 128],
                            hb[:, t * 128:(t + 1) * 128], ident_b[:])
                    dst = hT[buf][:, (k * ETC + g * 4) * 128:
                                  (k * ETC + g * 4 + 4) * 128]
                    if g % 2 == 0:
                        nc.vector.tensor_copy(dst, ptr[:])
                    else:
                        nc.scalar.activation(
                            dst, ptr[:], mybir.ActivationFunctionType.Copy)
            if dma_only:
                return
            # evacuate edge partial sums -> [e, 65] tiles -> DRAM bounce
            ar_in = arinp.tile([128, ETC * W], f32, tag="arin")
            for j in range(CC // 512):
                et = evacp.tile([W, 512], f32, tag="et")
                nc.vector.tensor_copy(et[:], pchs[j][:])
                for q in range(4):
                    ptr2 = ptr2p.tile([128, W], f32, tag="ptr2")
                    nc.tensor.transpose(
                        ptr2[:], et[:, q * 128:(q + 1) * 128],
                        ident[0:W, 0:W])
                    nc.vector.tensor_copy(
                        ar_in[:, (j * 4 + q) * W:(j * 4 + q + 1) * W],
                        ptr2[:])
            nc.scalar.dma_start(bounce_in[cc][:], ar_in[:])
            if SKIP_COLLECTIVE:
                nc.sync.dma_start(bounce_out[cc][:], bounce_in[cc][:])
            else:
                nc.gpsimd.collective_compute(
                    "AllReduce", mybir.AluOpType.add,
                    replica_groups=[list(range(N_CORES))],
                    ins=[bounce_in[cc].opt()], outs=[bounce_out[cc].opt()])

        def stage_b(cc, pools):
            pop = pools['pop']
            buf = cc % 2
            eftf = eftp.tile([128, ETC * W], f32, tag="eftf")
            nc.scalar.dma_start(eftf[:], bounce_out[cc][:])
            eft1 = eftp.tile([128, ETC * W], bf16, tag="eft1")
            for t in range(ETC):
                b = t * W
                de = eftp.tile([128, 1], f32, tag="de")
                nc.vector.tensor_scalar_max(
                    de[:], eftf[:, b + OUT_CH:b + W], 1e-20)
                rec = eftp.tile([128, 1], f32, tag="rec")
                nc.vector.reciprocal(rec[:], de[:])
                nc.vector.tensor_scalar_mul(
                    eft1[:, b:b + OUT_CH], eftf[:, b:b + OUT_CH], rec[:])
                nc.vector.memset(eft1[:, b + OUT_CH:b + W], 1.0)
            for k in range(NT):
                po = pop.tile([128, W], f32, tag="po")
                for t in range(ETC):
                    nc.tensor.matmul(
                        po[:],
                        hT[buf][:, (k * ETC + t) * 128:(k * ETC + t + 1) * 128],
                        eft1[:, t * W:(t + 1) * W],
                        start=(t == 0), stop=(t == ETC - 1))
                dst = po_acc[:, k * W:(k + 1) * W]
                if cc == 0:
                    if k % 2 == 0:
                        nc.vector.tensor_copy(dst, po[:])
                    else:
                        nc.scalar.activation(
                            dst, po[:], mybir.ActivationFunctionType.Copy)
                else:
                    nc.vector.tensor_tensor(
                        dst, dst, po[:], mybir.AluOpType.add)
                if cc == NCC - 1:
                    b = k * W
                    dn = eftp.tile([128, 1], f32, tag="dn")
                    nc.vector.tensor_scalar_max(
                        dn[:], po_acc[:, b + OUT_CH:b + W], 1e-20)
                    rcn = eftp.tile([128, 1], f32, tag="rcn")
                    nc.vector.reciprocal(rcn[:], dn[:])
                    ot = eftp.tile([128, OUT_CH], f32, tag="ot")
                    nc.vector.tensor_scalar_mul(
                        ot[:], po_acc[:, b:b + OUT_CH], rcn[:])
                    nc.scalar.dma_start(
                        out_ext[k * 128:(k + 1) * 128, :], ot[:])

        with (
            tc.tile_pool(name=f"pch{sfx}", bufs=1, space="PSUM") as pchp,
            tc.tile_pool(name=f"ptr{sfx}", bufs=2, space="PSUM") as ptrp,
            tc.tile_pool(name=f"ptr2{sfx}", bufs=1, space="PSUM") as ptr2p,
            tc.tile_pool(name=f"pop{sfx}", bufs=2, space="PSUM") as pop,
        ):
            pools = dict(pchp=pchp, ptrp=ptrp, ptr2p=ptr2p, pop=pop)
            for cc in range(NCC):
                stage_a(cc, pools)
                if not dma_only and cc >= 1:
                    stage_b(cc - 1, pools)
            if not dma_only:
                stage_b(NCC - 1, pools)

        # ---- epilogue: only needed for dma_only (out fused into last B) ----
        if dma_only:
            with tc.tile_pool(name=f"fin{sfx}", bufs=1) as finp:
                ot = finp.tile([128, OUT_CH], f32, tag="ot")
                nc.vector.memset(ot[:], 0.0)
                nc.vector.tensor_tensor(
                    ot[:, 0:1], ot[:, 0:1], acc[:], mybir.AluOpType.add)
                nc.sync.dma_start(out_ext[0:128, :], ot[:])


def build_graph(reps=1, dma_only=False):
    nc = bacc.Bacc("TRN2", target_bir_lowering=False, debug=False,
                   num_devices=N_CORES)
    x_ext = nc.dram_tensor("x", [NL, IN_CH], f32, kind="ExternalInput")
    H_ext = nc.dram_tensor("H", [NL, EP], f32, kind="ExternalInput")
    th_ext = nc.dram_tensor("theta", [IN_CH, OUT_CH], f32, kind="ExternalInput")
    id_ext = nc.dram_tensor("ident", [128, 128], f32, kind="ExternalInput")
    out_ext = nc.dram_tensor("out", [NL, OUT_CH], f32, kind="ExternalOutput")
    with tile.TileContext(nc) as tc:
        for r in range(reps):
            build_body(nc, tc, x_ext, H_ext, th_ext, id_ext, out_ext,
                       sfx=str(r), dma_only=dma_only)
    nc.compile()
    return nc


def make_in_maps(x, H, theta):
    x_pad = np.zeros((NL * N_CORES, IN_CH), np.float32)
    x_pad[:N] = x
    H_pad = np.zeros((NL * N_CORES, EP), np.float32)
    H_pad[:N, :E] = H
    ident = np.eye(128, dtype=np.float32)
    theta = np.asarray(theta, np.float32)
    in_maps = []
    for c in range(N_CORES):
        in_maps.append({
            "x": x_pad[c * NL:(c + 1) * NL],
            "H": H_pad[c * NL:(c + 1) * NL],
            "theta": theta,
            "ident": ident,
        })
    return in_maps


def kernel(x, H, theta):
    x = np.asarray(x, np.float32)
    H = np.asarray(H, np.float32)
    nc = build_graph(reps=1)
    in_maps = make_in_maps(x, H, theta)
    res = run_bass_kernel_spmd(nc, in_maps, core_ids=list(range(N_CORES)))
    out = np.concatenate(
        [res.results[c]["out"] for c in range(N_CORES)], axis=0)
    return np.ascontiguousarray(out[:N])


# revision 4
# speedup vs baseline: 3.9394x; 1.4619x over previous
"""DAHH hypergraph conv v2: single H read, fused transpose production,
chunked pipelined AllReduce.

reference:
    xp      = x @ theta                      [N, 64]
    de      = colsum(H)                      [E]
    edge_ft = (H.T @ xp) / de[:, None]       [E, 64]
    dn      = rowsum(H)                      [N]
    node_ft = (H @ edge_ft) / dn[:, None]    [N, 64]

Sharding: H and x row-sharded (node dim) across 8 cores; theta replicated.

v2 structure (vs baseline's 5 serial phases): the edge dim is split into
NCC=10 chunks of CC=1024. Per chunk cc, stage A loads H[:, cc] once
(f32 HWDGE), casts to bf16, and uses the SBUF-resident tiles TWICE:
(1) edge partial sums  pch = xp1.T @ hb  (psum [65, 512] x2), and
(2) PE transposes of every [128,128] subtile -> hT chunk buffer (bf16),
which stage B consumes as matmul weights — so H is read from HBM ONCE
(~105 MB/core instead of 210 MB). Each chunk's partial edge sums are
AllReduced independently (10 x 266KB collectives); stage B(cc-1) is
issued after stage A(cc) so each collective hides behind the next
chunk's compute. Node aggregation accumulates po (psum per chunk) into
an SBUF accumulator, normalized once at the end.

N padded 20000->20480 (2560/core), E padded 10000->10240 with zeros;
padding is numerically inert (zero rows/cols, degree clamps).
"""
import sys
sys.path.insert(0, "/opt/trn_rl_repo")
import numpy as np

import concourse.bass as bass
import concourse.bacc as bacc
import concourse.tile as tile
import concourse.mybir as mybir
from concourse.bass_utils import run_bass_kernel_spmd

N, E, IN_CH, OUT_CH = 20000, 10000, 128, 64
N_CORES = 8
NL = 2560            # padded nodes per core
EP = 10240           # padded edges
NT = NL // 128       # 20 node tiles per core
CC = 1024            # e-chunk width
NCC = EP // CC       # 10 chunks
ETC = CC // 128      # 8 e-tiles per chunk
W = 65               # 64 features + degree/ones column

f32 = mybir.dt.float32
bf16 = mybir.dt.bfloat16

SKIP_COLLECTIVE = False


def build_body(nc, tc, x_ext, H_ext, th_ext, id_ext, out_ext, sfx="",
               dma_only=False):
    with (
        tc.tile_pool(name=f"const{sfx}", bufs=1) as constp,
        tc.tile_pool(name=f"persist{sfx}", bufs=1) as persist,
        tc.tile_pool(name=f"dram{sfx}", bufs=1, space="DRAM") as dram,
        tc.tile_pool(name=f"hpool{sfx}", bufs=4) as hpool,
        tc.tile_pool(name=f"hbpool{sfx}", bufs=3) as hbpool,
        tc.tile_pool(name=f"evac{sfx}", bufs=2) as evacp,
        tc.tile_pool(name=f"eft{sfx}", bufs=2) as eftp,
        tc.tile_pool(name=f"arin{sfx}", bufs=2) as arinp,
    ):
        ident = constp.tile([128, 128], f32)
        nc.scalar.dma_start(ident[:], id_ext[:])
        ident_b = constp.tile([128, 128], bf16)
        nc.vector.tensor_copy(ident_b[:], ident[:])
        th_f = constp.tile([128, OUT_CH], f32)
        nc.scalar.dma_start(th_f[:], th_ext[:])
        th_b = constp.tile([128, OUT_CH], bf16)
        nc.vector.tensor_copy(th_b[:], th_f[:])
        acc = constp.tile([128, 1], f32)   # dma_only sink
        nc.vector.memset(acc[:], 0.0)

        xp1 = persist.tile([128, NT * W], bf16)
        hT = [persist.tile([128, ETC * NT * 128], bf16, name=f"hT{i}{sfx}")
              for i in range(2)]
        po_acc = persist.tile([128, NT * W], f32)
        bounce_in = [dram.tile([128, ETC * W], bf16, name=f"bin{i}{sfx}")
                     for i in range(NCC)]
        bounce_out = [dram.tile([128, ETC * W], bf16, addr_space="Shared",
                                name=f"bout{i}{sfx}")
                      for i in range(NCC)]

        # ---- phase 0: xp1 = [x @ theta | 1] per node tile ----
        with (
            tc.tile_pool(name=f"p0{sfx}", bufs=3) as p0,
            tc.tile_pool(name=f"p0ps{sfx}", bufs=2, space="PSUM") as p0ps,
        ):
            for k in range(NT):
                xt = p0.tile([128, 128], f32, tag="x")
                nc.scalar.dma_start(xt[:], x_ext[k * 128:(k + 1) * 128, :])
                pt = p0ps.tile([128, 128], f32, tag="pt")
                nc.tensor.transpose(pt[:], xt[:], ident[:])
                xT = p0.tile([128, 128], bf16, tag="xT")
                nc.vector.tensor_copy(xT[:], pt[:])
                pxp = p0ps.tile([128, OUT_CH], f32, tag="pxp")
                nc.tensor.matmul(pxp[:], xT[:], th_b[:], start=True, stop=True)
                nc.vector.tensor_copy(xp1[:, k * W:k * W + OUT_CH], pxp[:])
                nc.vector.memset(xp1[:, k * W + OUT_CH:(k + 1) * W], 1.0)

        def stage_a(cc, pools):
            pchp, ptrp, ptr2p = pools['pchp'], pools['ptrp'], pools['ptr2p']
            buf = cc % 2
            pchs = [pchp.tile([W, 512], f32, tag=f"pch{j}",
                              name=f"pch{j}_{cc}{sfx}")
                    for j in range(CC // 512)]
            for k in range(NT):
                h = hpool.tile([128, CC], f32, tag="h")
                nc.sync.dma_start(
                    h[:], H_ext[k * 128:(k + 1) * 128, cc * CC:(cc + 1) * CC])
                if dma_only:
                    nc.vector.tensor_tensor(
                        acc[:], acc[:], h[:, 0:1], mybir.AluOpType.add)
                    continue
                hb = hbpool.tile([128, CC], bf16, tag="hb")
                if k % 2 == 0:
                    nc.vector.tensor_copy(hb[:], h[:])
                else:
                    nc.scalar.activation(
                        hb[:], h[:], mybir.ActivationFunctionType.Copy)
                for j in range(CC // 512):
                    nc.tensor.matmul(
                        pchs[j][:],
                        xp1[:, k * W:(k + 1) * W],
                        hb[:, j * 512:(j + 1) * 512],
                        start=(k == 0), stop=(k == NT - 1),
                        skip_group_check=True)
                # transposes for stage B: hT[buf] tile (k, t) at col
                # (k*ETC+t)*128, grouped 4 per [128, 512] bf16 psum
                for g in range(ETC // 4):
                    ptr = ptrp.tile([128, 512], bf16, tag="ptr")
                    for q in range(4):
                        t = g * 4 + q
                        nc.tensor.transpose(
                            ptr[:, q * 128:(q + 1) * 128],
                            hb[:, t * 128:(t + 1) * 128], ident_b[:])
                    dst = hT[buf][:, (k * ETC + g * 4) * 128:
                                  (k * ETC + g * 4 + 4) * 128]
                    if g % 2 == 0:
                        nc.vector.tensor_copy(dst, ptr[:])
                    else:
                        nc.scalar.activation(
                            dst, ptr[:], mybir.ActivationFunctionType.Copy)
            if dma_only:
                return
            # evacuate edge partial sums -> [e, 65] tiles -> DRAM bounce
            ar_in = arinp.tile([128, ETC * W], bf16, tag="arin")
            for j in range(CC // 512):
                et = evacp.tile([W, 512], f32, tag="et")
                nc.vector.tensor_copy(et[:], pchs[j][:])
                for q in range(4):
                    ptr2 = ptr2p.tile([128, W], f32, tag="ptr2")
                    nc.tensor.transpose(
                        ptr2[:], et[:, q * 128:(q + 1) * 128],
                        ident[0:W, 0:W])
                    nc.vector.tensor_copy(
                        ar_in[:, (j * 4 + q) * W:(j * 4 + q + 1) * W],
                        ptr2[:])
            nc.scalar.dma_start(bounce_in[cc][:], ar_in[:])
            if SKIP_COLLECTIVE:
                nc.sync.dma_start(bounce_out[cc][:], bounce_in[cc][:])
            else:
                nc.gpsimd.collective_compute(
                    "AllReduce", mybir.AluOpType.add,
                    replica_groups=[list(range(N_CORES))],
                    ins=[bounce_in[cc].opt()], outs=[bounce_out[cc].opt()])

        def stage_b(cc, pools):
            pop = pools['pop']
            buf = cc % 2
            eftf = eftp.tile([128, ETC * W], bf16, tag="eftf")
            nc.scalar.dma_start(eftf[:], bounce_out[cc][:])
            eft1 = eftp.tile([128, ETC * W], bf16, tag="eft1")
            for t in range(ETC):
                b = t * W
                de = eftp.tile([128, 1], f32, tag="de")
                nc.vector.tensor_scalar_max(
                    de[:], eftf[:, b + OUT_CH:b + W], 1e-20)
                rec = eftp.tile([128, 1], f32, tag="rec")
                nc.vector.reciprocal(rec[:], de[:])
                nc.vector.tensor_scalar_mul(
                    eft1[:, b:b + OUT_CH], eftf[:, b:b + OUT_CH], rec[:])
                nc.vector.memset(eft1[:, b + OUT_CH:b + W], 1.0)
            for k in range(NT):
                po = pop.tile([128, W], f32, tag="po")
                for t in range(ETC):
                    nc.tensor.matmul(
                        po[:],
                        hT[buf][:, (k * ETC + t) * 128:(k * ETC + t + 1) * 128],
                        eft1[:, t * W:(t + 1) * W],
                        start=(t == 0), stop=(t == ETC - 1))
                dst = po_acc[:, k * W:(k + 1) * W]
                if cc == 0:
                    if k % 2 == 0:
                        nc.vector.tensor_copy(dst, po[:])
                    else:
                        nc.scalar.activation(
                            dst, po[:], mybir.ActivationFunctionType.Copy)
                else:
                    nc.vector.tensor_tensor(
                        dst, dst, po[:], mybir.AluOpType.add)
                if cc == NCC - 1:
                    b = k * W
                    dn = eftp.tile([128, 1], f32, tag="dn")
                    nc.vector.tensor_scalar_max(
                        dn[:], po_acc[:, b + OUT_CH:b + W], 1e-20)
                    rcn = eftp.tile([128, 1], f32, tag="rcn")
                    nc.vector.reciprocal(rcn[:], dn[:])
                    ot = eftp.tile([128, OUT_CH], f32, tag="ot")
                    nc.vector.tensor_scalar_mul(
                        ot[:], po_acc[:, b:b + OUT_CH], rcn[:])
                    nc.scalar.dma_start(
                        out_ext[k * 128:(k + 1) * 128, :], ot[:])

        with (
            tc.tile_pool(name=f"pch{sfx}", bufs=1, space="PSUM") as pchp,
            tc.tile_pool(name=f"ptr{sfx}", bufs=2, space="PSUM") as ptrp,
            tc.tile_pool(name=f"ptr2{sfx}", bufs=1, space="PSUM") as ptr2p,
            tc.tile_pool(name=f"pop{sfx}", bufs=2, space="PSUM") as pop,
        ):
            pools = dict(pchp=pchp, ptrp=ptrp, ptr2p=ptr2p, pop=pop)
            for cc in range(NCC):
                stage_a(cc, pools)
                if not dma_only and cc >= 1:
                    stage_b(cc - 1, pools)
            if not dma_only:
                stage_b(NCC - 1, pools)

        # ---- epilogue: only needed for dma_only (out fused into last B) ----
        if dma_only:
            with tc.tile_pool(name=f"fin{sfx}", bufs=1) as finp:
                ot = finp.tile([128, OUT_CH], f32, tag="ot")
                nc.vector.memset(ot[:], 0.0)
                nc.vector.tensor_tensor(
                    ot[:, 0:1], ot[:, 0:1], acc[:], mybir.AluOpType.add)
                nc.sync.dma_start(out_ext[0:128, :], ot[:])


def build_graph(reps=1, dma_only=False):
    nc = bacc.Bacc("TRN2", target_bir_lowering=False, debug=False,
                   num_devices=N_CORES)
    x_ext = nc.dram_tensor("x", [NL, IN_CH], f32, kind="ExternalInput")
    H_ext = nc.dram_tensor("H", [NL, EP], f32, kind="ExternalInput")
    th_ext = nc.dram_tensor("theta", [IN_CH, OUT_CH], f32, kind="ExternalInput")
    id_ext = nc.dram_tensor("ident", [128, 128], f32, kind="ExternalInput")
    out_ext = nc.dram_tensor("out", [NL, OUT_CH], f32, kind="ExternalOutput")
    with tile.TileContext(nc) as tc:
        for r in range(reps):
            build_body(nc, tc, x_ext, H_ext, th_ext, id_ext, out_ext,
                       sfx=str(r), dma_only=dma_only)
    nc.compile()
    return nc


def make_in_maps(x, H, theta):
    x_pad = np.zeros((NL * N_CORES, IN_CH), np.float32)
    x_pad[:N] = x
    H_pad = np.zeros((NL * N_CORES, EP), np.float32)
    H_pad[:N, :E] = H
    ident = np.eye(128, dtype=np.float32)
    theta = np.asarray(theta, np.float32)
    in_maps = []
    for c in range(N_CORES):
        in_maps.append({
            "x": x_pad[c * NL:(c + 1) * NL],
            "H": H_pad[c * NL:(c + 1) * NL],
            "theta": theta,
            "ident": ident,
        })
    return in_maps


def kernel(x, H, theta):
    x = np.asarray(x, np.float32)
    H = np.asarray(H, np.float32)
    nc = build_graph(reps=1)
    in_maps = make_in_maps(x, H, theta)
    res = run_bass_kernel_spmd(nc, in_maps, core_ids=list(range(N_CORES)))
    out = np.concatenate(
        [res.results[c]["out"] for c in range(N_CORES)], axis=0)
    return np.ascontiguousarray(out[:N])
